# revision 3
# baseline (speedup 1.0000x reference)
"""PointPillarScatter on 8 NeuronCores.

Full inputs -> full (B, C, NX, NY) output.

Sharding: core k handles (sample b = k//2, output-x half h = k%2), i.e. each
core produces out[b, :, h*216:(h+1)*216, :]  (flip along x baked into the
host-built gather indices).

Per-core device pipeline (9 chunks of 24 output-x rows each):
  1. dma_gather: canvas rows for 11904 positions from a compacted per-core
     pillar table in DRAM (row 0 = zeros for empty positions)
     -> SBUF [128 part (pos within block), 93 blocks, 64 ch]
  2. PE transpose per pair of 128-position blocks ([128,128] -> PSUM [128,128])
  3. DVE/ACT copy PSUM -> SBUF out tile [64 ch, 11904 pos]
  4. one 3 MB DMA out to the (C, X, Y) canvas slice in DRAM
"""

import sys

sys.path.insert(0, "/opt/trn_rl_repo")

import numpy as np

import concourse.bacc as bacc
import concourse.mybir as mybir
from concourse.bass_utils import run_bass_kernel_spmd
from concourse.masks import make_identity
from concourse.tile import TileContext

C = 64
NX = 432
NY = 496
B = 4
NCORES = 8
XH = NX // 2            # 216 x-rows per core
M = XH * NY             # 107136 positions per core
NF = 12032              # compacted feats rows per core (row 0 = zeros)

XCHUNK = 24             # x-rows per chunk
NCHUNK = XH // XCHUNK   # 9
MC = XCHUNK * NY        # 11904 positions per chunk
JBLK = MC // 128        # 93 blocks of 128 positions
IDXW = M // 16          # 6696 wrapped-index columns
IDXWC = MC // 16        # 744 per chunk

_CACHE = {}
LAST_RESULTS = None


def _build_program():
    nc = bacc.Bacc(None, target_bir_lowering=False)
    feats = nc.dram_tensor("feats", [NF, C], mybir.dt.float32, kind="ExternalInput")
    idx = nc.dram_tensor("idx", [128, IDXW], mybir.dt.int16, kind="ExternalInput")
    out = nc.dram_tensor("out", [C, XH, NY], mybir.dt.float32, kind="ExternalOutput")

    with TileContext(nc) as tc:
        with (
            tc.tile_pool(name="const", bufs=1) as constp,
            tc.tile_pool(name="gather", bufs=2) as gatherp,
            tc.tile_pool(name="idxp", bufs=2) as idxp,
            tc.tile_pool(name="outp", bufs=2) as outp,
            tc.tile_pool(name="psum", bufs=6, space="PSUM") as psump,
        ):
            ident = constp.tile([128, 128], mybir.dt.float32)
            make_identity(nc, ident[:])

            for ci in range(NCHUNK):
                it = idxp.tile([128, IDXWC], mybir.dt.int16, tag="it")
                nc.sync.dma_start(it[:], idx[:, ci * IDXWC:(ci + 1) * IDXWC])

                gt = gatherp.tile([128, JBLK * C], mybir.dt.float32, tag="gt")
                nc.gpsimd.dma_gather(
                    out_ap=gt[:].rearrange("p (j c) -> p j c", c=C),
                    in_ap=feats[:],
                    idxs_ap=it[:],
                    num_idxs=MC,
                    num_idxs_reg=MC,
                    elem_size=C,
                    single_packet=False,
                )

                ot = outp.tile([C, MC], mybir.dt.float32, tag="ot")
                npairs = JBLK // 2
                for k in range(npairs):
                    pt = psump.tile([128, 128], mybir.dt.float32, tag="pt")
                    nc.tensor.transpose(pt[:], gt[:, k * 128:(k + 1) * 128], ident[:])
                    if k % 3 == 2:
                        nc.scalar.copy(ot[:, (2 * k) * 128:(2 * k + 1) * 128], pt[0:64, :])
                        nc.scalar.copy(ot[:, (2 * k + 1) * 128:(2 * k + 2) * 128], pt[64:128, :])
                    else:
                        nc.vector.tensor_copy(ot[:, (2 * k) * 128:(2 * k + 1) * 128], pt[0:64, :])
                        nc.vector.tensor_copy(ot[:, (2 * k + 1) * 128:(2 * k + 2) * 128], pt[64:128, :])
                if JBLK % 2:
                    j = JBLK - 1
                    pt = psump.tile([128, 128], mybir.dt.float32, tag="pt")
                    nc.tensor.transpose(pt[0:64, :], gt[:, j * C:(j + 1) * C], ident[:])
                    nc.vector.tensor_copy(ot[:, j * 128:(j + 1) * 128], pt[0:64, :])

                nc.sync.dma_start(out[:, ci * XCHUNK:(ci + 1) * XCHUNK, :], ot[:].rearrange("c (x y) -> c x y", y=NY))

    nc.finalize()
    return nc


def _prep_in_maps(feats_full, batch_indices, sample_indices):
    x = batch_indices[:, 2].astype(np.int64)
    y = batch_indices[:, 1].astype(np.int64)
    s = sample_indices.astype(np.int64)
    xo = (NX - 1) - x
    h = xo // XH
    xl = xo % XH
    pos = xl * NY + y
    core = s * 2 + h

    in_maps = []
    for k in range(NCORES):
        rows = np.nonzero(core == k)[0]
        p = pos[rows]
        order = np.argsort(p)
        rows = rows[order]
        p = p[order]
        n = rows.size
        assert n + 1 <= NF
        fe = np.zeros((NF, C), np.float32)
        fe[1:n + 1] = feats_full[rows]
        idx_full = np.zeros(M, np.int16)
        idx_full[p] = np.arange(1, n + 1, dtype=np.int16)
        wrapped = idx_full.reshape(IDXW, 16).T          # [16, IDXW]
        idx_arr = np.tile(wrapped, (8, 1))              # replicate per Q7 core
        in_maps.append({"feats": fe, "idx": np.ascontiguousarray(idx_arr)})
    return in_maps


def kernel(batch_pillar_features, batch_indices, sample_indices, batch_size):
    global LAST_RESULTS
    if "nc" not in _CACHE:
        _CACHE["nc"] = _build_program()
    nc = _CACHE["nc"]

    feats_full = np.asarray(batch_pillar_features, np.float32)
    batch_indices = np.asarray(batch_indices)
    sample_indices = np.asarray(sample_indices)
    bs = int(batch_size)
    assert bs == B and feats_full.shape == (B * 12000, C)

    in_maps = _prep_in_maps(feats_full, batch_indices, sample_indices)
    res = run_bass_kernel_spmd(nc, in_maps, core_ids=list(range(NCORES)))
    LAST_RESULTS = res

    full = np.empty((B, C, NX, NY), np.float32)
    for k in range(NCORES):
        b, hh = k // 2, k % 2
        full[b, :, hh * XH:(hh + 1) * XH, :] = res.results[k]["out"]
    return full


# revision 4
# speedup vs baseline: 7.0871x; 7.0871x over previous
"""PointPillarScatter on 8 NeuronCores.

Full inputs -> full (B, C, NX, NY) float32 output.

Sharding: core k handles (sample b = k//2, output-x half h = k%2); each core
produces out[b, :, h*216:(h+1)*216, :] (the flip along x is baked into the
host-built scatter offsets).

Per-core device pipeline, two phases:

  Phase 1 (sparse scatter, ~6k rows/core):
    The ~6k real pillar rows are DMA'd densely into SBUF and scattered by
    dma_scatter_add into a runtime-pre-zeroed DRAM staging canvas.  Staging is
    laid out partition-major: partition p owns 838 consecutive rows (837 canvas
    positions {i : i % 128 == p} ordered by i // 128, plus 1 dump row for the
    padding slots), so the offsets bake in both the scatter and the
    transpose-friendly permutation, and int16 offsets stay in range per
    32-partition region.

  Phase 2 (dense stream, memory-bound):
    Per chunk of 24 output-x rows: one big contiguous DMA pulls the staging
    slice into SBUF as [128 pos-in-block, 93 blocks, 64 ch]; PE transposes
    pairs of 128-position blocks through an identity ([128,128] -> PSUM);
    DVE/ACT copy PSUM into the [64 ch, 11904 pos] out tile; one 3 MB DMA
    writes the (C, X, Y) canvas slice.
"""

import sys

sys.path.insert(0, "/opt/trn_rl_repo")

import numpy as np

import concourse.bacc as bacc
import concourse.mybir as mybir
from concourse.bass_utils import run_bass_kernel_spmd
from concourse.masks import make_identity
from concourse.tile import TileContext

C = 64
NX = 432
NY = 496
B = 4
NCORES = 8
XH = NX // 2            # 216 x-rows per core
M = XH * NY             # 107136 positions per core
P = 128
JPP = M // P            # 837 real rows per partition
RPP = JPP + 1           # +1 dump row
NREG = 4                # int16 offset range => scatter per 32-partition region
PREG = P // NREG        # 32 partitions per region
REGROWS = PREG * RPP    # 26816 staging rows per region

XCHUNK = 24
NCHUNK = XH // XCHUNK   # 9
MC = XCHUNK * NY        # 11904 positions per chunk
JBLK = MC // P          # 93 blocks of 128 positions

_CACHE = {}
LAST_RESULTS = None


def _build_program(jr):
    nslot = P * jr          # scatter slots per region (padded, fixed count)
    nc = bacc.Bacc(None, target_bir_lowering=False)
    feats = nc.dram_tensor("feats", [NREG * nslot, C], mybir.dt.float32, kind="ExternalInput")
    sidx = nc.dram_tensor("sidx", [P, NREG * nslot // 16], mybir.dt.int16, kind="ExternalInput")
    staging = nc.dram_tensor("staging", [P * RPP, C], mybir.dt.float32, kind="ExternalOutput")
    out = nc.dram_tensor("out", [C, XH, NY], mybir.dt.float32, kind="ExternalOutput")

    with TileContext(nc) as tc:
        with (
            tc.tile_pool(name="scat", bufs=2) as scatp,
            tc.tile_pool(name="sidxp", bufs=2) as sidxp,
        ):
            for r in range(NREG):
                ft = scatp.tile([P, jr, C], mybir.dt.float32, tag="ft")
                nc.sync.dma_start(ft[:], feats[r * nslot:(r + 1) * nslot, :].rearrange("(p j) c -> p j c", j=jr))
                it = sidxp.tile([P, nslot // 16], mybir.dt.int16, tag="it")
                nc.sync.dma_start(it[:], sidx[:, r * (nslot // 16):(r + 1) * (nslot // 16)])
                nc.gpsimd.dma_scatter_add(
                    out_ap=staging[r * REGROWS:(r + 1) * REGROWS, :],
                    in_ap=ft[:],
                    idxs_ap=it[:],
                    num_idxs=nslot,
                    num_idxs_reg=nslot,
                    elem_size=C,
                    single_packet=False,
                )

    stview = staging[:].rearrange("(pt j) c -> pt j c", j=RPP)   # [128, 838, 64]

    with TileContext(nc) as tc:
        with (
            tc.tile_pool(name="const", bufs=1) as constp,
            tc.tile_pool(name="gather", bufs=2) as gatherp,
            tc.tile_pool(name="outp", bufs=2) as outp,
            tc.tile_pool(name="psum", bufs=4, space="PSUM") as psump,
            tc.tile_pool(name="psums", bufs=2, space="PSUM") as psumsp,
        ):
            ident = constp.tile([P, P], mybir.dt.float32)
            make_identity(nc, ident[:])

            for ci in range(NCHUNK):
                gt = gatherp.tile([P, JBLK * C], mybir.dt.float32, tag="gt")
                nc.sync.dma_start(
                    gt[:].rearrange("p (j c) -> p j c", c=C),
                    stview[:, ci * JBLK:(ci + 1) * JBLK, :],
                )

                ot = outp.tile([C, MC], mybir.dt.float32, tag="ot")
                npairs = JBLK // 2          # 46
                nquads = (npairs + 3) // 4  # 12 (last quad has 2 pairs)
                for q in range(nquads):
                    np_q = min(4, npairs - q * 4)
                    pt = psump.tile([P, 512], mybir.dt.float32, tag="pt")
                    for m in range(np_q):
                        k = q * 4 + m
                        nc.tensor.transpose(pt[:, m * P:(m + 1) * P], gt[:, k * P:(k + 1) * P], ident[:])
                    # even-parity blocks (psum partitions 0:64), odd (64:128)
                    base = q * 4 * 2 * P    # starting column in ot
                    dst = ot[:, base:base + np_q * 2 * P].rearrange("c (n two x) -> c n two x", two=2, x=P)
                    src = pt[:, :np_q * P]
                    nc.vector.tensor_copy(dst[:, :, 0, :], src[0:C, :].rearrange("c (n x) -> c n x", x=P))
                    nc.scalar.copy(dst[:, :, 1, :], src[C:P, :].rearrange("c (n x) -> c n x", x=P))
                # leftover single block 92
                j = JBLK - 1
                pt = psumsp.tile([P, P], mybir.dt.float32, tag="pts")
                nc.tensor.transpose(pt[0:C, :], gt[:, j * C:(j + 1) * C], ident[:])
                nc.vector.tensor_copy(ot[:, j * P:(j + 1) * P], pt[0:C, :])

                nc.sync.dma_start(out[:, ci * XCHUNK:(ci + 1) * XCHUNK, :], ot[:].rearrange("c (x y) -> c x y", y=NY))

    nc.finalize()
    return nc


def _prep_in_maps(feats_full, batch_indices, sample_indices):
    x = batch_indices[:, 2].astype(np.int64)
    y = batch_indices[:, 1].astype(np.int64)
    s = sample_indices.astype(np.int64)
    xo = (NX - 1) - x
    h = xo // XH
    xl = xo % XH
    pos = xl * NY + y
    core = s * 2 + h

    pp = pos % P            # partition
    jj = pos // P           # row within partition
    reg = pp // PREG
    local = (pp % PREG) * RPP + jj   # int16-safe (< 26816)

    # fixed slot count per region, sized to the worst (core, region)
    maxn = 0
    for k in range(NCORES):
        for r in range(NREG):
            maxn = max(maxn, int(np.sum((core == k) & (reg == r))))
    jr = -(-(maxn + 1) // P) + 1     # ceil to 128 slots + 1 spare column

    nslot = P * jr
    in_maps = []
    for k in range(NCORES):
        feats_arr = np.zeros((NREG * nslot, C), np.float32)
        idx_arr = np.full((16, NREG * nslot // 16), 0, np.int16)
        for r in range(NREG):
            rows = np.nonzero((core == k) & (reg == r))[0]
            loc = local[rows]
            order = np.argsort(loc)
            rows = rows[order]
            loc = loc[order]
            n = rows.size
            assert n <= nslot
            slots = np.arange(nslot)
            vals = np.full(nslot, 0, np.int16)
            vals[:n] = loc.astype(np.int16)
            vals[n:] = ((slots[n:] % P) % PREG) * RPP + JPP   # dump row, zero source
            # slot s lives at dram row (s % 128) * jr + s // 128 within the region
            d = (slots[:n] % P) * jr + slots[:n] // P
            feats_arr[r * nslot + d] = feats_full[rows]
            # wrapped int16 layout: value for slot s at [s % 16, s // 16]
            idx_arr[:, r * (nslot // 16):(r + 1) * (nslot // 16)] = vals.reshape(nslot // 16, 16).T
        in_maps.append({"feats": feats_arr, "sidx": np.ascontiguousarray(np.tile(idx_arr, (8, 1)))})
    return in_maps, jr


def kernel(batch_pillar_features, batch_indices, sample_indices, batch_size):
    global LAST_RESULTS
    feats_full = np.asarray(batch_pillar_features, np.float32)
    batch_indices = np.asarray(batch_indices)
    sample_indices = np.asarray(sample_indices)
    bs = int(batch_size)
    assert bs == B and feats_full.shape[1] == C

    in_maps, jr = _prep_in_maps(feats_full, batch_indices, sample_indices)
    if _CACHE.get("jr") != jr:
        _CACHE["nc"] = _build_program(jr)
        _CACHE["jr"] = jr
    nc = _CACHE["nc"]

    res = run_bass_kernel_spmd(nc, in_maps, core_ids=list(range(NCORES)))
    LAST_RESULTS = res

    full = np.empty((B, C, NX, NY), np.float32)
    for k in range(NCORES):
        b, hh = k // 2, k % 2
        full[b, :, hh * XH:(hh + 1) * XH, :] = res.results[k]["out"]
    return full


# revision 5
# speedup vs baseline: 7.1936x; 1.0150x over previous
"""PointPillarScatter on 8 NeuronCores.

Full inputs -> full (B, C, NX, NY) float32 output.

Sharding: core k handles (sample b = k//2, output-x half h = k%2); each core
produces out[b, :, h*216:(h+1)*216, :] (the flip along x is baked into the
host-built scatter offsets).

Per-core device pipeline, two phases:

  Phase 1 (sparse scatter, ~6k rows/core):
    The ~6k real pillar rows are DMA'd densely into SBUF and scattered by
    dma_scatter_add into a runtime-pre-zeroed DRAM staging canvas.  Staging is
    laid out partition-major: partition p owns 838 consecutive rows (837 canvas
    positions {i : i % 128 == p} ordered by i // 128, plus 1 dump row for the
    padding slots), so the offsets bake in both the scatter and the
    transpose-friendly permutation, and int16 offsets stay in range per
    32-partition region.

  Phase 2 (dense stream, memory-bound):
    Per chunk of 24 output-x rows: one big contiguous DMA pulls the staging
    slice into SBUF as [128 pos-in-block, 93 blocks, 64 ch]; PE transposes
    pairs of 128-position blocks through an identity ([128,128] -> PSUM);
    DVE/ACT copy PSUM into the [64 ch, 11904 pos] out tile; one 3 MB DMA
    writes the (C, X, Y) canvas slice.
"""

import sys

sys.path.insert(0, "/opt/trn_rl_repo")

import numpy as np

import concourse.bacc as bacc
import concourse.mybir as mybir
from concourse.bass_utils import run_bass_kernel_spmd
from concourse.masks import make_identity
from concourse.tile import TileContext

C = 64
NX = 432
NY = 496
B = 4
NCORES = 8
XH = NX // 2            # 216 x-rows per core
M = XH * NY             # 107136 positions per core
P = 128
JPP = M // P            # 837 real rows per partition
RPP = JPP + 1           # +1 dump row
NREG = 4                # int16 offset range => scatter per 32-partition region
PREG = P // NREG        # 32 partitions per region
REGROWS = PREG * RPP    # 26816 staging rows per region

XCHUNK = 24
NCHUNK = XH // XCHUNK   # 9
MC = XCHUNK * NY        # 11904 positions per chunk
JBLK = MC // P          # 93 blocks of 128 positions

_CACHE = {}
LAST_RESULTS = None


def _build_program(jr):
    nslot = P * jr          # scatter slots per region (padded, fixed count)
    nc = bacc.Bacc(None, target_bir_lowering=False)
    feats = nc.dram_tensor("feats", [NREG * nslot, C], mybir.dt.float32, kind="ExternalInput")
    sidx = nc.dram_tensor("sidx", [P, NREG * nslot // 16], mybir.dt.int16, kind="ExternalInput")
    staging = nc.dram_tensor("staging", [P * RPP, C], mybir.dt.float32, kind="ExternalOutput")
    out = nc.dram_tensor("out", [C, XH, NY], mybir.dt.float32, kind="ExternalOutput")

    with TileContext(nc) as tc:
        with (
            tc.tile_pool(name="scat", bufs=2) as scatp,
            tc.tile_pool(name="sidxp", bufs=2) as sidxp,
        ):
            for r in range(NREG):
                ft = scatp.tile([P, jr, C], mybir.dt.float32, tag="ft")
                nc.scalar.dma_start(ft[:], feats[r * nslot:(r + 1) * nslot, :].rearrange("(p j) c -> p j c", j=jr))
                it = sidxp.tile([P, nslot // 16], mybir.dt.int16, tag="it")
                nc.scalar.dma_start(it[:], sidx[:, r * (nslot // 16):(r + 1) * (nslot // 16)])
                nc.gpsimd.dma_scatter_add(
                    out_ap=staging[r * REGROWS:(r + 1) * REGROWS, :],
                    in_ap=ft[:],
                    idxs_ap=it[:],
                    num_idxs=nslot,
                    num_idxs_reg=nslot,
                    elem_size=C,
                    single_packet=False,
                )

    stview = staging[:].rearrange("(pt j) c -> pt j c", j=RPP)   # [128, 838, 64]

    with TileContext(nc) as tc:
        with (
            tc.tile_pool(name="const", bufs=1) as constp,
            tc.tile_pool(name="gather", bufs=2) as gatherp,
            tc.tile_pool(name="outp", bufs=2) as outp,
            tc.tile_pool(name="psum", bufs=4, space="PSUM") as psump,
            tc.tile_pool(name="psums", bufs=2, space="PSUM") as psumsp,
        ):
            ident = constp.tile([P, P], mybir.dt.float32)
            make_identity(nc, ident[:])

            for ci in range(NCHUNK):
                gt = gatherp.tile([P, JBLK * C], mybir.dt.float32, tag="gt")
                nc.scalar.dma_start(
                    gt[:].rearrange("p (j c) -> p j c", c=C),
                    stview[:, ci * JBLK:(ci + 1) * JBLK, :],
                )

                ot = outp.tile([C, MC], mybir.dt.float32, tag="ot")
                npairs = JBLK // 2          # 46
                nquads = (npairs + 3) // 4  # 12 (last quad has 2 pairs)
                for q in range(nquads):
                    np_q = min(4, npairs - q * 4)
                    pt = psump.tile([P, 512], mybir.dt.float32, tag="pt")
                    for m in range(np_q):
                        k = q * 4 + m
                        nc.tensor.transpose(pt[:, m * P:(m + 1) * P], gt[:, k * P:(k + 1) * P], ident[:])
                    # even-parity blocks (psum partitions 0:64), odd (64:128)
                    base = q * 4 * 2 * P    # starting column in ot
                    dst = ot[:, base:base + np_q * 2 * P].rearrange("c (n two x) -> c n two x", two=2, x=P)
                    src = pt[:, :np_q * P]
                    nc.vector.tensor_copy(dst[:, :, 0, :], src[0:C, :].rearrange("c (n x) -> c n x", x=P))
                    nc.scalar.copy(dst[:, :, 1, :], src[C:P, :].rearrange("c (n x) -> c n x", x=P))
                # leftover single block 92
                j = JBLK - 1
                pt = psumsp.tile([P, P], mybir.dt.float32, tag="pts")
                nc.tensor.transpose(pt[0:C, :], gt[:, j * C:(j + 1) * C], ident[:])
                nc.vector.tensor_copy(ot[:, j * P:(j + 1) * P], pt[0:C, :])

                nc.sync.dma_start(out[:, ci * XCHUNK:(ci + 1) * XCHUNK, :], ot[:].rearrange("c (x y) -> c x y", y=NY))

    nc.finalize()
    return nc


def _prep_in_maps(feats_full, batch_indices, sample_indices):
    x = batch_indices[:, 2].astype(np.int64)
    y = batch_indices[:, 1].astype(np.int64)
    s = sample_indices.astype(np.int64)
    xo = (NX - 1) - x
    h = xo // XH
    xl = xo % XH
    pos = xl * NY + y
    core = s * 2 + h

    pp = pos % P            # partition
    jj = pos // P           # row within partition
    reg = pp // PREG
    local = (pp % PREG) * RPP + jj   # int16-safe (< 26816)

    # fixed slot count per region, sized to the worst (core, region)
    maxn = 0
    for k in range(NCORES):
        for r in range(NREG):
            maxn = max(maxn, int(np.sum((core == k) & (reg == r))))
    jr = -(-(maxn + 1) // P) + 1     # ceil to 128 slots + 1 spare column

    nslot = P * jr
    in_maps = []
    for k in range(NCORES):
        feats_arr = np.zeros((NREG * nslot, C), np.float32)
        idx_arr = np.full((16, NREG * nslot // 16), 0, np.int16)
        for r in range(NREG):
            rows = np.nonzero((core == k) & (reg == r))[0]
            loc = local[rows]
            order = np.argsort(loc)
            rows = rows[order]
            loc = loc[order]
            n = rows.size
            assert n <= nslot
            slots = np.arange(nslot)
            vals = np.full(nslot, 0, np.int16)
            vals[:n] = loc.astype(np.int16)
            vals[n:] = ((slots[n:] % P) % PREG) * RPP + JPP   # dump row, zero source
            # slot s lives at dram row (s % 128) * jr + s // 128 within the region
            d = (slots[:n] % P) * jr + slots[:n] // P
            feats_arr[r * nslot + d] = feats_full[rows]
            # wrapped int16 layout: value for slot s at [s % 16, s // 16]
            idx_arr[:, r * (nslot // 16):(r + 1) * (nslot // 16)] = vals.reshape(nslot // 16, 16).T
        in_maps.append({"feats": feats_arr, "sidx": np.ascontiguousarray(np.tile(idx_arr, (8, 1)))})
    return in_maps, jr


def kernel(batch_pillar_features, batch_indices, sample_indices, batch_size):
    global LAST_RESULTS
    feats_full = np.asarray(batch_pillar_features, np.float32)
    batch_indices = np.asarray(batch_indices)
    sample_indices = np.asarray(sample_indices)
    bs = int(batch_size)
    assert bs == B and feats_full.shape[1] == C

    in_maps, jr = _prep_in_maps(feats_full, batch_indices, sample_indices)
    if _CACHE.get("jr") != jr:
        _CACHE["nc"] = _build_program(jr)
        _CACHE["jr"] = jr
    nc = _CACHE["nc"]

    res = run_bass_kernel_spmd(nc, in_maps, core_ids=list(range(NCORES)))
    LAST_RESULTS = res

    full = np.empty((B, C, NX, NY), np.float32)
    for k in range(NCORES):
        b, hh = k // 2, k % 2
        full[b, :, hh * XH:(hh + 1) * XH, :] = res.results[k]["out"]
    return full


# revision 6
# speedup vs baseline: 8.8470x; 1.2299x over previous
"""PointPillarScatter on 8 NeuronCores.

Full inputs -> full (B, C, NX, NY) float32 output.

Sharding: core k handles (sample b = k//2, output-x half h = k%2); each core
produces out[b, :, h*216:(h+1)*216, :] (the flip along x is baked into the
host-built scatter offsets).

Per-core device pipeline, two phases:

  Phase 1 (sparse scatter, ~6k rows/core):
    The ~6k real pillar rows are DMA'd densely into SBUF and scattered by
    dma_scatter_add into a runtime-pre-zeroed DRAM staging canvas.  Staging is
    laid out partition-major: partition p owns 838 consecutive rows (837 canvas
    positions {i : i % 128 == p} ordered by i // 128, plus 1 dump row for the
    padding slots), so the offsets bake in both the scatter and the
    transpose-friendly permutation, and int16 offsets stay in range per
    32-partition region.

  Phase 2 (dense stream, memory-bound):
    Per chunk of 24 output-x rows: one big contiguous DMA pulls the staging
    slice into SBUF as [128 pos-in-block, 93 blocks, 64 ch]; PE transposes
    pairs of 128-position blocks through an identity ([128,128] -> PSUM);
    DVE/ACT copy PSUM into the [64 ch, 11904 pos] out tile; one 3 MB DMA
    writes the (C, X, Y) canvas slice.
"""

import sys

sys.path.insert(0, "/opt/trn_rl_repo")

import numpy as np

import concourse.bacc as bacc
import concourse.mybir as mybir
from concourse.bass_utils import run_bass_kernel_spmd
from concourse.masks import make_identity
from concourse.tile import TileContext

C = 64
NX = 432
NY = 496
B = 4
NCORES = 8
XH = NX // 2            # 216 x-rows per core
M = XH * NY             # 107136 positions per core
P = 128
JPP = M // P            # 837 real rows per partition
RPP = JPP + 1           # +1 dump row
NREG = 4                # int16 offset range => scatter per 32-partition region
PREG = P // NREG        # 32 partitions per region
REGROWS = PREG * RPP    # 26816 staging rows per region

XCHUNK = 8
NCHUNK = XH // XCHUNK   # 27
MC = XCHUNK * NY        # 3968 positions per chunk
JBLK = MC // P          # 31 blocks of 128 positions

_CACHE = {}
LAST_RESULTS = None


def _build_program(jr):
    nslot = P * jr          # scatter slots per region (padded, fixed count)
    nc = bacc.Bacc(None, target_bir_lowering=False)
    feats = nc.dram_tensor("feats", [NREG * nslot, C], mybir.dt.float32, kind="ExternalInput")
    sidx = nc.dram_tensor("sidx", [P, NREG * nslot // 16], mybir.dt.int16, kind="ExternalInput")
    staging = nc.dram_tensor("staging", [P * RPP, C], mybir.dt.float32, kind="ExternalOutput")
    out = nc.dram_tensor("out", [C, XH, NY], mybir.dt.float32, kind="ExternalOutput")

    with TileContext(nc) as tc:
        with (
            tc.tile_pool(name="scat", bufs=2) as scatp,
            tc.tile_pool(name="sidxp", bufs=2) as sidxp,
        ):
            for r in range(NREG):
                ft = scatp.tile([P, jr, C], mybir.dt.float32, tag="ft")
                nc.scalar.dma_start(ft[:], feats[r * nslot:(r + 1) * nslot, :].rearrange("(p j) c -> p j c", j=jr))
                it = sidxp.tile([P, nslot // 16], mybir.dt.int16, tag="it")
                nc.scalar.dma_start(it[:], sidx[:, r * (nslot // 16):(r + 1) * (nslot // 16)])
                nc.gpsimd.dma_scatter_add(
                    out_ap=staging[r * REGROWS:(r + 1) * REGROWS, :],
                    in_ap=ft[:],
                    idxs_ap=it[:],
                    num_idxs=nslot,
                    num_idxs_reg=nslot,
                    elem_size=C,
                    single_packet=False,
                )

    stview = staging[:].rearrange("(pt j) c -> pt j c", j=RPP)   # [128, 838, 64]

    with TileContext(nc) as tc:
        with (
            tc.tile_pool(name="const", bufs=1) as constp,
            tc.tile_pool(name="gather", bufs=4) as gatherp,
            tc.tile_pool(name="outp", bufs=3) as outp,
            tc.tile_pool(name="psum", bufs=4, space="PSUM") as psump,
            tc.tile_pool(name="psums", bufs=2, space="PSUM") as psumsp,
        ):
            ident = constp.tile([P, P], mybir.dt.float32)
            make_identity(nc, ident[:])

            for ci in range(NCHUNK):
                gt = gatherp.tile([P, JBLK * C], mybir.dt.float32, tag="gt")
                nc.scalar.dma_start(
                    gt[:].rearrange("p (j c) -> p j c", c=C),
                    stview[:, ci * JBLK:(ci + 1) * JBLK, :],
                )

                ot = outp.tile([C, MC], mybir.dt.float32, tag="ot")
                npairs = JBLK // 2          # 46
                nquads = (npairs + 3) // 4  # 12 (last quad has 2 pairs)
                for q in range(nquads):
                    np_q = min(4, npairs - q * 4)
                    pt = psump.tile([P, 512], mybir.dt.float32, tag="pt")
                    for m in range(np_q):
                        k = q * 4 + m
                        nc.tensor.transpose(pt[:, m * P:(m + 1) * P], gt[:, k * P:(k + 1) * P], ident[:])
                    # even-parity blocks (psum partitions 0:64), odd (64:128)
                    base = q * 4 * 2 * P    # starting column in ot
                    dst = ot[:, base:base + np_q * 2 * P].rearrange("c (n two x) -> c n two x", two=2, x=P)
                    src = pt[:, :np_q * P]
                    nc.vector.tensor_copy(dst[:, :, 0, :], src[0:C, :].rearrange("c (n x) -> c n x", x=P))
                    nc.scalar.copy(dst[:, :, 1, :], src[C:P, :].rearrange("c (n x) -> c n x", x=P))
                # leftover single block 92
                j = JBLK - 1
                pt = psumsp.tile([P, P], mybir.dt.float32, tag="pts")
                nc.tensor.transpose(pt[0:C, :], gt[:, j * C:(j + 1) * C], ident[:])
                nc.vector.tensor_copy(ot[:, j * P:(j + 1) * P], pt[0:C, :])

                nc.sync.dma_start(out[:, ci * XCHUNK:(ci + 1) * XCHUNK, :], ot[:].rearrange("c (x y) -> c x y", y=NY))

    nc.finalize()
    return nc


def _prep_in_maps(feats_full, batch_indices, sample_indices):
    x = batch_indices[:, 2].astype(np.int64)
    y = batch_indices[:, 1].astype(np.int64)
    s = sample_indices.astype(np.int64)
    xo = (NX - 1) - x
    h = xo // XH
    xl = xo % XH
    pos = xl * NY + y
    core = s * 2 + h

    pp = pos % P            # partition
    jj = pos // P           # row within partition
    reg = pp // PREG
    local = (pp % PREG) * RPP + jj   # int16-safe (< 26816)

    # fixed slot count per region, sized to the worst (core, region)
    maxn = 0
    for k in range(NCORES):
        for r in range(NREG):
            maxn = max(maxn, int(np.sum((core == k) & (reg == r))))
    jr = -(-(maxn + 1) // P) + 1     # ceil to 128 slots + 1 spare column

    nslot = P * jr
    in_maps = []
    for k in range(NCORES):
        feats_arr = np.zeros((NREG * nslot, C), np.float32)
        idx_arr = np.full((16, NREG * nslot // 16), 0, np.int16)
        for r in range(NREG):
            rows = np.nonzero((core == k) & (reg == r))[0]
            loc = local[rows]
            order = np.argsort(loc)
            rows = rows[order]
            loc = loc[order]
            n = rows.size
            assert n <= nslot
            slots = np.arange(nslot)
            vals = np.full(nslot, 0, np.int16)
            vals[:n] = loc.astype(np.int16)
            vals[n:] = ((slots[n:] % P) % PREG) * RPP + JPP   # dump row, zero source
            # slot s lives at dram row (s % 128) * jr + s // 128 within the region
            d = (slots[:n] % P) * jr + slots[:n] // P
            feats_arr[r * nslot + d] = feats_full[rows]
            # wrapped int16 layout: value for slot s at [s % 16, s // 16]
            idx_arr[:, r * (nslot // 16):(r + 1) * (nslot // 16)] = vals.reshape(nslot // 16, 16).T
        in_maps.append({"feats": feats_arr, "sidx": np.ascontiguousarray(np.tile(idx_arr, (8, 1)))})
    return in_maps, jr


def kernel(batch_pillar_features, batch_indices, sample_indices, batch_size):
    global LAST_RESULTS
    feats_full = np.asarray(batch_pillar_features, np.float32)
    batch_indices = np.asarray(batch_indices)
    sample_indices = np.asarray(sample_indices)
    bs = int(batch_size)
    assert bs == B and feats_full.shape[1] == C

    in_maps, jr = _prep_in_maps(feats_full, batch_indices, sample_indices)
    if _CACHE.get("jr") != jr:
        _CACHE["nc"] = _build_program(jr)
        _CACHE["jr"] = jr
    nc = _CACHE["nc"]

    res = run_bass_kernel_spmd(nc, in_maps, core_ids=list(range(NCORES)))
    LAST_RESULTS = res

    full = np.empty((B, C, NX, NY), np.float32)
    for k in range(NCORES):
        b, hh = k // 2, k % 2
        full[b, :, hh * XH:(hh + 1) * XH, :] = res.results[k]["out"]
    return full
